# revision 1
# baseline (speedup 1.0000x reference)
"""Multi-head attention (B=4, N=2048, C=1024, H=16) on 8 Trainium2 NeuronCores.

Sharding: data parallel over batch (4-way) x tensor parallel over heads
(2-way, 8 heads per group). Core c handles batch c//2 and head group c%2.
Each core computes a partial projection output [2048, 1024]; the host sums
the two head-group partials per batch and adds b_proj.

Per-core kernel (all matmuls in fp32r = full-rate ~fp22 precision):
  P1  x [2048,1024] -> xT [1024,2048] via PE transposes
  P3  V = xT^T @ Wv in natural [tokens, 64] layout per head (+ ones column
      appended, so the PV matmul also accumulates softmax denominators)
  P2  Q^T, K^T = Wqk^T @ xT in [feature, token] layout
  P4  per head: S^T = K @ Q^T (two heads packed in the PE array via
      tile_position row groups), P^T = exp(S^T * scale) on ScalarE,
      O'^T = [V|1]^T @ P^T accumulated over key chunks in PSUM,
      normalize rows 0..63 by reciprocal of row 64 (broadcast via DMA)
  P5  y_partial = O^T(all heads) @ Wproj rows for this group
"""

import numpy as np

B, N, C, H, D = 4, 2048, 1024, 16, 64
SCALE = float(D) ** -0.5
T = 2048          # tokens per core (one batch)
G = 2             # head groups
HL = H // G       # 8 local heads
NP = HL // 2      # 4 head pairs
F = HL * D        # 512 local features
VW = 66           # V tile column stride per head (64 data + 1 ones + 1 pad)
NCORES = 8


P4_MODE = "full"  # full | noqk | nopv | copyexp


def _build_bass(reps=1, phases="12345", p4_mode=None):
    p4_mode = p4_mode or P4_MODE
    import concourse.bacc as bacc
    import concourse.tile as tile
    from concourse import mybir
    import concourse.bass as bass
    f32 = mybir.dt.float32
    f32r = mybir.dt.float32r
    EXP = mybir.ActivationFunctionType.Exp

    nc = bacc.Bacc("TRN2", debug=False, num_devices=NCORES)
    xt_d = nc.dram_tensor("xt", [C, T], f32, kind="ExternalInput")
    wqkv_d = nc.dram_tensor("wqkv", [C, 3 * F], f32, kind="ExternalInput")
    wproj_d = nc.dram_tensor("wproj", [F, C], f32, kind="ExternalInput")
    ones_d = nc.dram_tensor("ones", [128, HL], f32, kind="ExternalInput")
    zeros_d = nc.dram_tensor("zeros", [64, T], f32, kind="ExternalInput")
    y_d = nc.dram_tensor("y", [T, C], f32, kind="ExternalOutput")

    xtr = xt_d.ap().bitcast(f32r)
    wqkvr = wqkv_d.ap().bitcast(f32r)
    wprojr = wproj_d.ap().bitcast(f32r)

    # alternate PSUM->SBUF copies between DVE and ACT to balance engine load
    def eng_copy(i, out, in_):
        if i % 2 == 0:
            nc.vector.tensor_copy(out, in_)
        else:
            nc.scalar.copy(out, in_)

    with tile.TileContext(nc) as tc:
        with (
            tc.tile_pool(name="consts", bufs=1) as consts,
            tc.tile_pool(name="qkt", bufs=1) as qkt_pool,
            tc.tile_pool(name="vsb", bufs=1) as v_pool,
        ):
            ones_sb = consts.tile([128, HL], f32)
            nc.sync.dma_start(out=ones_sb, in_=ones_d.ap())

            for _rep in range(reps):
                QKT = [qkt_pool.tile([128, T], f32r, tag=f"qkt{m}", name=f"qkt{m}")
                       for m in range(8)]
                V = [v_pool.tile([128, HL * VW], f32r, tag=f"v{t}", name=f"v{t}")
                     for t in range(16)]

                with (
                    tc.tile_pool(name="xT", bufs=1) as xT_pool,
                    tc.tile_pool(name="psmm", bufs=2, space="PSUM") as psmm,
                ):
                    XT = [xT_pool.tile([128, T], f32r, tag=f"xT{c}", name=f"xT{c}")
                          for c in range(8)]

                    # -------- P3: V projection (xT loaded in column groups) ----
                    with tc.tile_pool(name="wv", bufs=1) as wv_pool:
                        WV = [wv_pool.tile([128, F], f32r, tag=f"wv{c}",
                                           name=f"wv{c}") for c in range(8)]
                        for c in range(8):
                            nc.sync.dma_start(
                                out=WV[c],
                                in_=wqkvr[c * 128:(c + 1) * 128, 2 * F:3 * F],
                            )
                        for tg in range(4):
                            for c in range(8):
                                nc.sync.dma_start(
                                    out=XT[c][:, tg * 512:(tg + 1) * 512],
                                    in_=xtr[c * 128:(c + 1) * 128,
                                            tg * 512:(tg + 1) * 512],
                                )
                            for t in range(4 * tg, 4 * tg + 4):
                                vp = psmm.tile([128, F], f32, tag="mm", name="vp")
                                for c in range(8):
                                    nc.tensor.matmul(
                                        vp,
                                        XT[c][:, t * 128:(t + 1) * 128],
                                        WV[c],
                                        start=(c == 0),
                                        stop=(c == 7),
                                    )
                                v3 = V[t].rearrange("p (h w) -> p h w", w=VW)
                                nc.vector.tensor_copy(
                                    v3[:, :, 64:65],
                                    ones_sb.rearrange("p (a b) -> p a b", b=1),
                                )
                                nc.vector.tensor_copy(
                                    v3[:, :, 0:64],
                                    vp.rearrange("p (h w) -> p h w", w=64),
                                )

                    # -------- P2: all Q^T/K^T projections --------------------
                    with tc.tile_pool(name="wqk", bufs=1) as wqk_pool:
                        WQK = [
                            wqk_pool.tile([128, 2 * F], f32r, tag=f"wqk{c}",
                                          name=f"wqk{c}")
                            for c in range(8)
                        ]
                        for c in range(8):
                            nc.sync.dma_start(
                                out=WQK[c],
                                in_=wqkvr[c * 128:(c + 1) * 128, 0:2 * F],
                            )
                        if "2" not in phases:
                            continue
                        ci = 0
                        for m in (NP, 0, NP + 1, 1, NP + 2, 2, NP + 3, 3):
                            for n in range(4):
                                qp = psmm.tile([128, 512], f32, tag="mm",
                                               name="qp")
                                for c in range(8):
                                    nc.tensor.matmul(
                                        qp,
                                        WQK[c][:, m * 128:(m + 1) * 128],
                                        XT[c][:, n * 512:(n + 1) * 512],
                                        start=(c == 0),
                                        stop=(c == 7),
                                    )
                                eng_copy(ci, QKT[m][:, n * 512:(n + 1) * 512],
                                         qp)
                                ci += 1

                # -------- P4 + P5 ------------------------------------------
                with tc.tile_pool(name="ot", bufs=1) as ot_pool:
                    with (
                        tc.tile_pool(name="ktpad", bufs=4) as ktpad_pool,
                        tc.tile_pool(name="pt", bufs=6) as pt_pool,
                        tc.tile_pool(name="nrm", bufs=1) as nrm_pool,
                        tc.tile_pool(name="ocp", bufs=2) as ocp_pool,
                        tc.tile_pool(name="dscr", bufs=4, space="DRAM") as dscr_pool,
                    ):
                            zr = zeros_d.ap().bitcast(f32r)

                            def build_ktpad(hp):
                                KTp = QKT[NP + hp]
                                tiles = []
                                for h in range(2):
                                    kp = ktpad_pool.tile([128, T], f32r, tag="ktp",
                                                         name="ktp")
                                    zrow = (1 - h) * 64
                                    nc.sync.dma_start(
                                        out=kp[zrow:zrow + 64, :], in_=zr)
                                    nc.sync.dma_start(
                                        out=kp[h * 64:(h + 1) * 64, :],
                                        in_=KTp[h * 64:(h + 1) * 64, :])
                                    tiles.append(kp)
                                return tiles

                            if "4" not in phases:
                                continue
                            OT = [ot_pool.tile([128, T], f32r, tag=f"ot{p}",
                                               name=f"ot{p}") for p in range(NP)]
                            with (
                                tc.tile_pool(name="ps_st", bufs=2, space="PSUM") as ps_st,
                                tc.tile_pool(name="ps_ot", bufs=2, space="PSUM") as ps_ot,
                            ):
                                ktpad_cur = build_ktpad(0)
                                for hp in range(NP):
                                    QTp = QKT[hp]
                                    KTpad = ktpad_cur
                                    for qh in range(2):
                                        q0 = qh * 1024
                                        otp = [ps_ot.tile([65, 1024], f32, tag="otp",
                                                          name="otp") for _ in range(2)]
                                        ptts = [None, None]
                                        stps = [None, None]

                                        def emit_qk(h, k):
                                            pts = []
                                            for nn in range(2):
                                                stq = ps_st.tile(
                                                    [128, 512], f32,
                                                    tag=f"st{h}", name=f"st{h}")
                                                nc.tensor.matmul(
                                                    stq,
                                                    KTpad[h][:, k * 128:(k + 1) * 128],
                                                    QTp[:,
                                                        q0 + nn * 512:q0 + (nn + 1) * 512],
                                                    start=True,
                                                    stop=True,
                                                )
                                                ptq = pt_pool.tile(
                                                    [128, 512], f32r,
                                                    tag=f"pt{h}", name=f"pt{h}")
                                                nc.scalar.activation(
                                                    ptq, stq, EXP, scale=SCALE)
                                                pts.append(ptq)
                                            ptts[h] = pts

                                        def emit_pv(h, k, pts):
                                            lh = 2 * hp + h
                                            for nn in range(2):
                                                nc.tensor.matmul(
                                                    otp[h][:, nn * 512:(nn + 1) * 512],
                                                    V[k][:, lh * VW:lh * VW + 65],
                                                    pts[nn],
                                                    start=(k == 0),
                                                    stop=(k == 15),
                                                )

                                        # software pipeline: PV lags QK/exp by one k-chunk so the
                                        # in-order PE queue never blocks the next QK behind a PV
                                        prev = [None, None]
                                        for k in range(16):
                                            for h in range(2):
                                                emit_qk(h, k)
                                                if prev[h] is not None:
                                                    emit_pv(h, k - 1, prev[h])
                                                prev[h] = ptts[h]
                                        for h in range(2):
                                            emit_pv(h, 15, prev[h])

                                        for h in range(2):
                                            # copy (incl. denominator row) to
                                            # SBUF so the PSUM accumulator is
                                            # released fast; the reciprocal
                                            # broadcast round trip then runs
                                            # off the critical path
                                            ocp = ocp_pool.tile([65, 1024], f32,
                                                                tag="ocp",
                                                                name="ocp")
                                            nc.vector.tensor_copy(ocp, otp[h])
                                            rec = nrm_pool.tile([1, 1024], f32, tag="rec", name="rec")
                                            nc.vector.reciprocal(rec, ocp[64:65, :])
                                            rec_d = dscr_pool.tile([1, 1024], f32, tag="rec_d",
                                                                   name="rec_d")
                                            nc.sync.dma_start(out=rec_d, in_=rec)
                                            rb = nrm_pool.tile([64, 1024], f32, tag="rb", name="rb")
                                            bcast = bass.AP(
                                                tensor=rec_d.tensor,
                                                offset=rec_d.offset,
                                                ap=[[0, 64]] + list(rec_d.ap[1:]),
                                            )
                                            nc.sync.dma_start(out=rb, in_=bcast)
                                            nc.vector.tensor_mul(
                                                OT[hp][h * 64:(h + 1) * 64, q0:q0 + 1024],
                                                ocp[0:64, :],
                                                rb,
                                            )
                                    if hp + 1 < NP:
                                        ktpad_cur = build_ktpad(hp + 1)

                    # ---------------- P5: output projection ----------------
                    if "5" not in phases:
                        continue
                    with (
                        tc.tile_pool(name="wp", bufs=1) as wp_pool,
                        tc.tile_pool(name="ysb", bufs=2) as y_pool,
                        tc.tile_pool(name="ps5", bufs=2, space="PSUM") as ps5,
                    ):
                        WP = [
                            wp_pool.tile([128, C], f32r, tag=f"wp{p}",
                                         name=f"wp{p}")
                            for p in range(NP)
                        ]
                        for p in range(NP):
                            nc.sync.dma_start(
                                out=WP[p],
                                in_=wprojr[p * 128:(p + 1) * 128, :]
                            )
                        ci = 0
                        for t in range(16):
                            for n in range(2):
                                yp = ps5.tile([128, 512], f32, tag="mm5",
                                              name="yp")
                                for hp in range(NP):
                                    nc.tensor.matmul(
                                        yp,
                                        OT[hp][:, t * 128:(t + 1) * 128],
                                        WP[hp][:, n * 512:(n + 1) * 512],
                                        start=(hp == 0),
                                        stop=(hp == NP - 1),
                                    )
                                yt = y_pool.tile([128, 512], f32, tag="yt",
                                                 name="yt")
                                eng_copy(ci, yt, yp)
                                ci += 1
                                nc.sync.dma_start(
                                    out=y_d.ap()[t * 128:(t + 1) * 128,
                                                 n * 512:(n + 1) * 512],
                                    in_=yt,
                                )

    nc.compile()
    return nc


_CACHE = {}


def _get_exec(reps=1):
    """Build + jit the 8-core SPMD executable once per process."""
    key = ("exec", reps)
    if key in _CACHE:
        return _CACHE[key]

    import jax
    from jax.experimental.shard_map import shard_map
    from jax.sharding import Mesh, PartitionSpec
    import concourse.mybir as mybir
    from concourse.bass2jax import (
        _bass_exec_p,
        install_neuronx_cc_hook,
        partition_id_tensor,
    )

    install_neuronx_cc_hook()
    nc = _build_bass(reps)

    partition_name = (
        nc.partition_id_tensor.name if nc.partition_id_tensor else None
    )
    in_names, out_names, out_avals, out_shapes = [], [], [], []
    for alloc in nc.m.functions[0].allocations:
        if not isinstance(alloc, mybir.MemoryLocationSet):
            continue
        name = alloc.memorylocations[0].name
        if alloc.kind == "ExternalInput":
            if name == partition_name:
                continue
            in_names.append(name)
        elif alloc.kind == "ExternalOutput":
            out_names.append(name)
            shape = tuple(alloc.tensor_shape)
            dtype = mybir.dt.np(alloc.dtype)
            out_avals.append(jax.core.ShapedArray(shape, dtype))
            out_shapes.append((shape, dtype))
    n_params = len(in_names)
    n_outs = len(out_names)
    all_names = in_names + out_names
    if partition_name is not None:
        all_names = all_names + [partition_name]

    def _body(*args):
        operands = list(args)
        if partition_name is not None:
            operands.append(partition_id_tensor())
        outs = _bass_exec_p.bind(
            *operands,
            out_avals=tuple(out_avals),
            in_names=tuple(all_names),
            out_names=tuple(out_names),
            lowering_input_output_aliases=(),
            sim_require_finite=True,
            sim_require_nnan=True,
            nc=nc,
        )
        return tuple(outs)

    devices = jax.devices()[:NCORES]
    mesh = Mesh(np.asarray(devices), ("core",))
    donate = tuple(range(n_params, n_params + n_outs))
    sharded = jax.jit(
        shard_map(
            _body,
            mesh=mesh,
            in_specs=(PartitionSpec("core"),) * (n_params + n_outs),
            out_specs=(PartitionSpec("core"),) * n_outs,
            check_rep=False,
        ),
        donate_argnums=donate,
        keep_unused=True,
    )
    _CACHE[key] = (sharded, in_names, out_names, out_shapes)
    return _CACHE[key]


def _shard_inputs(x, w_qkv, w_proj):
    """Per-core input dict, keyed by DRAM tensor name."""
    x = np.ascontiguousarray(np.asarray(x, dtype=np.float32))
    w_qkv = np.asarray(w_qkv, dtype=np.float32)
    w_proj = np.asarray(w_proj, dtype=np.float32)
    maps = []
    for c in range(NCORES):
        b, g = c // G, c % G
        wq = w_qkv[:, g * F:(g + 1) * F]
        wk = w_qkv[:, C + g * F:C + (g + 1) * F]
        wv = w_qkv[:, 2 * C + g * F:2 * C + (g + 1) * F]
        maps.append({
            "xt": np.ascontiguousarray(x[b].T),
            "wqkv": np.ascontiguousarray(np.concatenate([wq, wk, wv], axis=1)),
            "wproj": np.ascontiguousarray(w_proj[g * F:(g + 1) * F, :]),
            "ones": np.ones((128, HL), dtype=np.float32),
            "zeros": np.zeros((64, N), dtype=np.float32),
        })
    return maps


def _run_cores(in_maps):
    """Execute the SPMD program; returns list of per-core output dicts."""
    sharded, in_names, out_names, out_shapes = _get_exec()
    concat_in = [
        np.concatenate([m[name] for m in in_maps], axis=0) for name in in_names
    ]
    concat_zeros = [
        np.zeros((NCORES * s[0],) + tuple(s[1:]), dt) for s, dt in out_shapes
    ]
    out_arrs = sharded(*concat_in, *concat_zeros)
    outs = []
    for c in range(NCORES):
        outs.append({
            name: np.asarray(out_arrs[i]).reshape((NCORES,) + out_shapes[i][0])[c]
            for i, name in enumerate(out_names)
        })
    return outs


def kernel(x, w_qkv, w_proj, b_proj):
    in_maps = _shard_inputs(x, w_qkv, w_proj)
    outs = _run_cores(in_maps)
    b_proj = np.asarray(b_proj, dtype=np.float32)
    y = np.empty((B, N, C), dtype=np.float32)
    for b in range(B):
        y[b] = outs[G * b]["y"] + outs[G * b + 1]["y"] + b_proj
    return y


if __name__ == "__main__":
    # compile-only sanity check (bacc passes + full walrus codegen)
    import sys
    import tempfile
    import time as _time

    t0 = _time.time()
    nc = _build_bass()
    print(f"bacc build+compile OK ({_time.time()-t0:.1f}s)")
    if "--walrus" in sys.argv:
        from concourse.bass_utils import compile_bass_kernel

        t0 = _time.time()
        with tempfile.TemporaryDirectory() as td:
            compile_bass_kernel(nc, td)
        print(f"walrus compile OK ({_time.time()-t0:.1f}s)")

